# revision 35
# baseline (speedup 1.0000x reference)
"""Causal multi-head self-attention on 8 Trainium2 NeuronCores.

Sharding: data-parallel over batch (B=2) x tensor-parallel over heads
(16 heads -> 4 per core).  Each core computes, for its batch element and
its 4 heads: Q/K/V projections, causal softmax attention, and a partial
output projection (row-parallel Wo).  The host sums the 4 TP partials per
batch and adds bo.

Layout: the host passes x.T and pre-transposed weight shards so every
matmul contraction dim lands on SBUF partitions with no on-device
transposes:
  Q^T[o,t] = sum_d WqT[d,o] xT[d,t]     K^T likewise
  V[t,c]   = sum_d xT[d,t] WvT[d,c]
  S^T[j,i] = sum_c KT[c,j] QT[c,i]      per head, K=64 contraction
  P^T      = exp(S^T/sqrt(hd))          ACT applies the 1/8 scale for free
  ctx^T    = sum_j Vaug[j,:] P^T[j,i]   Vaug = [v(64) | ones(64)] so rows
                                        64..127 of ctx^T psum hold the
                                        softmax denominator (replicated),
                                        and the stationary is 128 wide
                                        (enables PE fast-weight-load)
  CT       = ctx^T[0:64] * recip(ctx^T[64:128])   one DVE recip + one mult
  out[t,o] = sum_c CT[c,t] WoT[c,o]

Q^T/K^T store head pairs stacked on partitions (head 2p rows 0-63, head
2p+1 rows 64-127, no zero padding).  One attention unit = one j-tile for
BOTH heads of a pair: the two K=64 S^T matmuls sit in disjoint PE row
groups (base partitions 0/64) and the hardware runs them concurrently
(~2x on the S stream); one batched ACTIVATE exps both heads' scores
([128, 2, 512] across 2 psum banks), amortizing the ~350-cycle ACT fixed
cost.  Causal masking multiplies the diagonal 128-col block by a static
triangular mask on the DVE; columns left of the diagonal are never
computed, exp'd, or read (trapezoid slicing).  S units are emitted two
units ahead of PV units (stride-2, giving same-shape runs of 4 matmuls)
so the PE never waits on ACT latency; projections of the next i-chunk
and the output projection of the previous i-chunk are interleaved into
the stream as fill work.  Exec: ~156-161us (baseline 191.8us).
"""
import math

import ml_dtypes
import numpy as np

import concourse.bass as bass
import concourse.mybir as mybir
import concourse.tile as tile
from concourse import bacc
from concourse.bass_utils import run_bass_kernel_spmd

F32 = mybir.dt.float32
BF16 = mybir.dt.bfloat16
AF = mybir.ActivationFunctionType
OP = mybir.AluOpType

B, T, D, H, HD = 2, 2048, 1024, 16, 64
NCORES, TP = 8, 4
HPC = H // TP          # heads per core = 4
CS = HPC * HD          # channel shard per core = 256
SCALE = 1.0 / math.sqrt(HD)
KB = D // 128          # 8 k-blocks of the d contraction
TCH = 512              # i-chunk (queries per attention inner pass)
NI = T // TCH          # 4 i-chunks
NTB = T // 128         # 16 token blocks
JGRP = 2               # j-tiles per S->exp->PV group (2 psum banks)
LA = 6                 # S-group lookahead ahead of PV in the PE stream

_CACHE: dict = {}
LAST_EXEC_NS = None
LAST_RESULTS = None


def _build(has_bias: bool, dbg: bool = False):
    nc = bacc.Bacc("TRN2", target_bir_lowering=False, debug=False,
                   num_devices=NCORES)

    xT_d = nc.dram_tensor("xT", [D, T], BF16, kind="ExternalInput").ap()
    wqT_d = nc.dram_tensor("wqT", [D, CS], BF16, kind="ExternalInput").ap()
    wkT_d = nc.dram_tensor("wkT", [D, CS], BF16, kind="ExternalInput").ap()
    wvT_d = nc.dram_tensor("wvT", [D, CS], BF16, kind="ExternalInput").ap()
    woT_d = nc.dram_tensor("woT", [CS, D], BF16, kind="ExternalInput").ap()
    bq_d = nc.dram_tensor("bq", [1, CS], BF16, kind="ExternalInput").ap()
    bk_d = nc.dram_tensor("bk", [1, CS], BF16, kind="ExternalInput").ap()
    bv_d = nc.dram_tensor("bv", [1, CS], BF16, kind="ExternalInput").ap()
    ones_d = nc.dram_tensor("ones", [1, T], BF16, kind="ExternalInput").ap()
    out_d = nc.dram_tensor("out", [T, D], BF16, kind="ExternalOutput").ap()

    with tile.TileContext(nc) as tc:
        with (
            tc.tile_pool(name="persist", bufs=1) as pp,
        ):
            # head-pair layout: pair p = heads 2p (rows 0-63) and 2p+1
            # (rows 64-127); no zero padding anywhere.
            QT = pp.tile([128, 2, T], BF16)
            KT = pp.tile([128, 2, T], BF16)
            # V augmented: [t-part, tb, h, c'] with c' 0-63 = ones, 64-127
            # = v channels -> PV psum rows 0-63 become the softmax
            # denominator (base partition 0, which the custom-DVE
            # reciprocal requires) and the PV stationary is 128 wide (FWL).
            V = pp.tile([128, NTB, HPC, 2 * HD], BF16)
            CT = pp.tile([128, 2, T], BF16)      # ctx^T (normalized)
            WO = pp.tile([128, 2, D], BF16)
            MK = pp.tile([128, 128], BF16)       # causal triangle: n >= p
            ONES = pp.tile([1, T], BF16)

            # ones columns of V (denominator rows of the PV psum); chunk-0
            # token blocks first so the prologue V pieces aren't blocked
            nc.gpsimd.memset(MK[:], 1.0)
            nc.gpsimd.affine_select(
                out=MK[:], in_=MK[:], compare_op=OP.is_ge, fill=0.0,
                base=0, channel_multiplier=-1, pattern=[[1, 128]])
            nc.gpsimd.memset(V[:, 0:4, :, 0:HD], 1.0)

            # ---------- pipelined projections + attention + out-proj ------
            with (
                tc.tile_pool(name="ph1", bufs=1) as p1,
                tc.tile_pool(name="pt", bufs=10) as ptp,
                tc.tile_pool(name="sm", bufs=2) as smp,
                tc.tile_pool(name="ost", bufs=4) as ostp,
                tc.tile_pool(name="pss", bufs=3, space="PSUM") as pss,
                tc.tile_pool(name="psc", bufs=2, space="PSUM") as psc,
            ):
                XT = p1.tile([128, KB, T], BF16)
                WQ = p1.tile([128, KB, CS], BF16)
                WK = p1.tile([128, KB, CS], BF16)
                WV = p1.tile([128, KB, CS], BF16)
                BQ = BK = BV = None
                if has_bias:
                    BQ = p1.tile([1, CS], BF16)
                    BK = p1.tile([1, CS], BF16)
                    BV = p1.tile([1, CS], BF16)

                # DMA: x streamed as (t-chunk, kb) tiles so chunk-0
                # projections (and attention) start early.  The scalar
                # queue only carries prologue weights -- DMA instructions
                # execute on the issuing engine's sequencer, and the
                # scalar engine must stay free for exp during attention.
                # x streamed per (t-chunk, kb): consumers wait on queue
                # counters, so critical transfers sit early on short
                # queues; chunk 0 spreads over all three DMA-capable queues
                xt_view = xT_d.rearrange("(a p) t -> a p t", p=128)
                nc.scalar.dma_start(out=WQ[:], in_=wqT_d.rearrange("(a p) c -> p a c", p=128))
                for tcn in range(NI):
                    tsl = slice(tcn * TCH, (tcn + 1) * TCH)
                    for kb in range(KB):
                        if tcn == 0:
                            eng = (nc.sync, nc.scalar, nc.gpsimd)[kb % 3]
                        else:
                            eng = nc.sync if kb % 2 == 0 else nc.scalar
                        eng.dma_start(out=XT[:, kb, tsl], in_=xt_view[kb][:, tsl])
                    if tcn == 0:
                        nc.sync.dma_start(out=WK[:], in_=wkT_d.rearrange("(a p) c -> p a c", p=128))
                        nc.scalar.dma_start(out=WV[:], in_=wvT_d.rearrange("(a p) c -> p a c", p=128))
                        if has_bias:
                            nc.scalar.dma_start(out=BQ[:], in_=bq_d[:])
                            nc.scalar.dma_start(out=BK[:], in_=bk_d[:])
                            nc.scalar.dma_start(out=BV[:], in_=bv_d[:])
                        nc.gpsimd.memset(V[:, 4:NTB, :, 0:HD], 1.0)
                    if tcn == 1:
                        nc.gpsimd.dma_start(out=WO[:], in_=woT_d.rearrange("(a p) o -> p a o", p=128))
                        nc.gpsimd.dma_start(out=ONES[:], in_=ones_d[:])

                def qk_piece(W_sb, bt, dst, ob, tcn):
                    tsl = slice(tcn * TCH, (tcn + 1) * TCH)
                    ps = pss.tile([128, JGRP, TCH], F32, tag="pss",
                                  name=f"pj{ob}_{tcn}")
                    for kb in range(KB):
                        nc.tensor.matmul(
                            ps[:, 0, :],
                            W_sb[:, kb, ob * 128:(ob + 1) * 128],
                            XT[:, kb, tsl],
                            start=(kb == 0),
                            stop=(kb == KB - 1 and not has_bias))
                    if has_bias:
                        nc.tensor.matmul(
                            ps[:, 0, :], bt[0:1, ob * 128:(ob + 1) * 128],
                            ONES[0:1, tsl], start=False, stop=True)
                    nc.vector.tensor_copy(out=dst[:, ob, tsl], in_=ps[:, 0, :])

                def v_piece(tb):
                    ps = pss.tile([128, JGRP, TCH], F32, tag="pss",
                                  name=f"pv{tb}")
                    for kb in range(KB):
                        nc.tensor.matmul(
                            ps[:, 0, 0:CS],
                            XT[:, kb, tb * 128:(tb + 1) * 128],
                            WV[:, kb, :],
                            start=(kb == 0),
                            stop=(kb == KB - 1 and not has_bias))
                    if has_bias:
                        nc.tensor.matmul(
                            ps[:, 0, 0:CS], ONES[0:1, tb * 128:(tb + 1) * 128],
                            BV[0:1, :], start=False, stop=True)
                    nc.vector.tensor_copy(
                        out=V[:, tb, :, HD:2 * HD],
                        in_=ps[:, 0, 0:CS].rearrange("p (h c) -> p h c", c=HD))

                def outproj_unit(tb, on):
                    ps = pss.tile([128, JGRP, TCH], F32, tag="pss",
                                  name=f"po{tb}_{on}")
                    for cbk in range(2):
                        nc.tensor.matmul(
                            ps[:, 0, :],
                            CT[:, cbk, tb * 128:(tb + 1) * 128],
                            WO[:, cbk, on * TCH:(on + 1) * TCH],
                            start=(cbk == 0), stop=(cbk == 1))
                    ob_sb = ostp.tile([128, TCH], BF16, tag="ost")
                    if tb >= (NI - 1) * 4:
                        # tail: ACT is free of exp by now -- split drains
                        if (tb + on) % 2 == 0:
                            nc.vector.tensor_copy(out=ob_sb[:], in_=ps[:, 0, :])
                        else:
                            nc.scalar.copy(out=ob_sb[:], in_=ps[:, 0, :])
                        eng = nc.sync if (tb + on) % 2 == 0 else nc.scalar
                    else:
                        nc.vector.tensor_copy(out=ob_sb[:], in_=ps[:, 0, :])
                        eng = nc.sync if (tb + on) % 2 == 0 else nc.gpsimd
                    eng.dma_start(
                        out=out_d[tb * 128:(tb + 1) * 128,
                                  on * TCH:(on + 1) * TCH],
                        in_=ob_sb[:])

                # prologue: chunk-0 projections
                for ob in range(2):
                    qk_piece(WQ, BQ, QT, ob, 0)
                for ob in range(2):
                    qk_piece(WK, BK, KT, ob, 0)
                for tb in range(4):
                    v_piece(tb)

                # flat software-pipelined attention unit stream.  One unit
                # = one j-tile for BOTH heads of a pair: the two S matmuls
                # contract K=64 in disjoint PE row groups (rows 0-63 /
                # 64-127) so the hardware runs them concurrently, and one
                # batched exp covers both.  Fill work (next-chunk
                # projections, prev-chunk out-proj) is injected after each
                # pair completes.
                units = []      # (icn, pr, jt, first, last)
                for icn in range(NI):
                    ntiles = (icn + 1) * (TCH // 128)
                    for pr in range(2):
                        for jt in range(ntiles):
                            units.append((icn, pr, jt,
                                          jt == 0, jt == ntiles - 1))

                def fills_for(icn, pr):
                    out = []
                    if pr == 0:
                        if icn < NI - 1:
                            out += [lambda ob=ob: qk_piece(WQ, BQ, QT, ob, icn + 1)
                                    for ob in range(2)]
                        if icn > 0:
                            out += [lambda tb=tb, on=on: outproj_unit(tb, on)
                                    for tb in range((icn - 1) * 4, (icn - 1) * 4 + 2)
                                    for on in range(2)]
                    else:
                        if icn < NI - 1:
                            out += [lambda ob=ob: qk_piece(WK, BK, KT, ob, icn + 1)
                                    for ob in range(2)]
                        if icn > 0:
                            out += [lambda tb=tb, on=on: outproj_unit(tb, on)
                                    for tb in range((icn - 1) * 4 + 2, (icn - 1) * 4 + 4)
                                    for on in range(2)]
                        if icn < NI - 1:
                            out += [lambda tb=tb: v_piece(tb)
                                    for tb in range((icn + 1) * 4, (icn + 1) * 4 + 4)]
                    return out

                pctx_of = {}
                pend = {}

                def emit_s(ui):
                    icn, pr, jt, first, last = units[ui]
                    i0 = icn * TCH
                    sc = 128 * (jt - icn * 4) if jt >= icn * 4 else 0
                    ps2 = pss.tile([128, 2, TCH], F32, tag="pss",
                                   name=f"ps{icn}_{pr}_{jt}")
                    pt2 = ptp.tile([128, 2, TCH], BF16, tag="pt",
                                   name=f"pt{icn}_{pr}_{jt}")
                    for hh in range(2):
                        po = 64 * hh
                        nc.tensor.matmul(
                            ps2[:, hh, sc:],
                            KT[po:po + 64, pr, jt * 128:(jt + 1) * 128],
                            QT[po:po + 64, pr, i0 + sc:i0 + TCH],
                            start=True, stop=True)
                    nc.scalar.activation(pt2[:, :, sc:], ps2[:, :, sc:],
                                         AF.Exp, scale=SCALE)
                    if jt >= icn * 4:
                        for hh in range(2):
                            nc.vector.tensor_tensor(
                                out=pt2[:, hh, sc:sc + 128],
                                in0=pt2[:, hh, sc:sc + 128],
                                in1=MK[:], op=OP.mult)
                    pend[ui] = (sc, pt2)

                def emit_pv(ui):
                    icn, pr, jt, first, last = units[ui]
                    ntiles = (icn + 1) * (TCH // 128)
                    i0 = icn * TCH
                    if first:
                        for hh in range(2):
                            pctx_of[(icn, pr, hh)] = psc.tile(
                                [128, TCH], F32, tag="psc",
                                name=f"pctx{icn}_{pr}_{hh}")
                    sc, pt2 = pend.pop(ui)
                    for hh in range(2):
                        nc.tensor.matmul(
                            pctx_of[(icn, pr, hh)][:, sc:],
                            V[:, jt, 2 * pr + hh, :],
                            pt2[:, hh, sc:],
                            start=(jt == 0), stop=(jt == ntiles - 1))
                    if last:
                        for hh in range(2):
                            pctx = pctx_of.pop((icn, pr, hh))
                            po = 64 * hh
                            rc = smp.tile([64, TCH], F32, tag="rc")
                            with nc.allow_low_precision(reason="softmax denom"):
                                nc.vector.reciprocal_approx_fast(
                                    out=rc[:], in_=pctx[0:64, :])
                            nc.vector.tensor_tensor(
                                out=CT[po:po + 64, pr, i0:i0 + TCH],
                                in0=pctx[64:128, :], in1=rc[:], op=OP.mult)
                        for f in fills_for(icn, pr):
                            f()

                # stride-2 emission: S,S then PV,PV -> same-shape runs of
                # 4 matmuls on the PE (fewer tile-config switches)
                assert len(units) % 2 == 0 and LA % 2 == 0
                for base in range(0, len(units), 2):
                    emit_s(base)
                    emit_s(base + 1)
                    if base >= LA:
                        emit_pv(base - LA)
                        emit_pv(base - LA + 1)
                for ui in range(len(units) - LA, len(units)):
                    emit_pv(ui)
                for tb in range((NI - 1) * 4, NI * 4):
                    for on in range(2):
                        outproj_unit(tb, on)

            if dbg:
                qt_o = nc.dram_tensor("qt_o", [128, 2, T], BF16,
                                      kind="ExternalOutput").ap()
                kt_o = nc.dram_tensor("kt_o", [128, 2, T], BF16,
                                      kind="ExternalOutput").ap()
                v_o = nc.dram_tensor("v_o", [128, NTB, HPC, 2 * HD], BF16,
                                     kind="ExternalOutput").ap()
                ct_o = nc.dram_tensor("ct_o", [128, 2, T], BF16,
                                      kind="ExternalOutput").ap()
                nc.sync.dma_start(out=qt_o[:], in_=QT[:])
                nc.sync.dma_start(out=kt_o[:], in_=KT[:])
                nc.sync.dma_start(out=v_o[:], in_=V[:])
                nc.sync.dma_start(out=ct_o[:], in_=CT[:])

    nc.compile()
    return nc


def _get_nc(has_bias: bool):
    key = ("nc", has_bias)
    if key not in _CACHE:
        _CACHE[key] = _build(has_bias)
    return _CACHE[key]


def _maybe_wire_ntff_hook():
    try:
        import antenv.axon_hooks  # noqa: F401  already present
        return
    except ImportError:
        pass
    try:
        import sys, types
        import trn_agent_boot.trn_boot as boot
        hook = boot._ntff_profile_via_ctypes("/opt/axon/libaxon_pjrt.so")
        mod = types.ModuleType("antenv.axon_hooks")
        mod.get_axon_ntff_profile_hook = lambda: hook
        mod.set_axon_ntff_profile_hook = lambda h: None
        sys.modules["antenv.axon_hooks"] = mod
    except Exception:
        pass


def kernel(x, Wq, bq, Wk, bk, Wv, bv, Wo, bo, _trace=False):
    global LAST_EXEC_NS, LAST_RESULTS
    x = np.asarray(x, np.float32)
    Wq = np.asarray(Wq, np.float32); bq = np.asarray(bq, np.float32)
    Wk = np.asarray(Wk, np.float32); bk = np.asarray(bk, np.float32)
    Wv = np.asarray(Wv, np.float32); bv = np.asarray(bv, np.float32)
    Wo = np.asarray(Wo, np.float32); bo = np.asarray(bo, np.float32)

    has_bias = bool(np.any(bq) or np.any(bk) or np.any(bv))
    nc = _get_nc(has_bias)

    BFNP = ml_dtypes.bfloat16
    ones = np.ones((1, T), BFNP)
    xTs = [np.ascontiguousarray(x[b].T).astype(BFNP) for b in range(B)]

    in_maps = []
    for c in range(NCORES):
        b, tpr = divmod(c, TP)
        rows = slice(CS * tpr, CS * (tpr + 1))
        in_maps.append({
            "xT": xTs[b],
            "wqT": np.ascontiguousarray(Wq[rows, :].T).astype(BFNP),
            "wkT": np.ascontiguousarray(Wk[rows, :].T).astype(BFNP),
            "wvT": np.ascontiguousarray(Wv[rows, :].T).astype(BFNP),
            "woT": np.ascontiguousarray(Wo[:, rows].T).astype(BFNP),
            "bq": np.ascontiguousarray(bq[rows]).reshape(1, CS).astype(BFNP),
            "bk": np.ascontiguousarray(bk[rows]).reshape(1, CS).astype(BFNP),
            "bv": np.ascontiguousarray(bv[rows]).reshape(1, CS).astype(BFNP),
            "ones": ones,
        })

    if _trace:
        _maybe_wire_ntff_hook()
    res = run_bass_kernel_spmd(nc, in_maps, core_ids=list(range(NCORES)),
                               trace=bool(_trace))
    LAST_EXEC_NS = res.exec_time_ns
    LAST_RESULTS = res

    out = np.empty((B, T, D), np.float32)
    for b in range(B):
        acc = res.results[TP * b]["out"].astype(np.float32)
        for tpr in range(1, TP):
            acc = acc + res.results[TP * b + tpr]["out"].astype(np.float32)
        out[b] = acc + bo[None, :]
    return out
